# revision 4
# baseline (speedup 1.0000x reference)
"""MoE routed expert matmul on 8 Trainium2 NeuronCores.

Problem: out[n] = input[n] @ w[inds[n]] + b[inds[n]]
  input [262144, 32] f32, inds [262144] i32 (1024 experts), w [1024, 32, 32], b [1024, 1, 32]

Strategy (K-stacked expert quads; host does routing/layout only — all FLOPs
on device):
  * Host sorts the 1024 experts by global token count (ascending) and chunks
    them into 32 quad-groups of 32 experts with near-equal counts.  Chunk q
    supplies one expert to each (core, band) pair: expert chunks[q][4k + r]
    goes to core k, quad q, band r.  Every core runs the same program over
    its own 32 quads; quad q's column width Q[q] = max token count in the
    chunk (global max, so the SPMD shapes match), rounded up to 8.  Because
    chunk members have near-equal counts, padding is only a few percent.
  * Activation layout xt [128, TOTW] fp16: token t of (quad q, band r) sits
    at column X[q] + t, rows 32r..32r+32 (its 32 features); the other 96 rows
    of that column are zero unless used by the other bands' tokens.  Each
    column carries up to 4 tokens (one per band) — full 128-row density.
  * Per quad, the kernel builds a block-diagonal [128, 128] fp16 lhsT on
    device: 4 tiny matmuls (identity lhsT, tile_position=(32r, 32r)) copy the
    4 experts' [32, 32] weights onto the diagonal of a pre-zeroed PSUM
    region, and one Vector/Scalar op copies it to SBUF.  Then ONE
    [K=128, M=128, N=Q] matmul per quad computes all 4 bands' tokens in a
    single pass — each activation column streams through the PE once (the
    off-diagonal zeros kill the cross-expert terms), 4x fewer PE rows than
    per-expert 32x32-tile matmuls.
  * The PSUM result + per-quad bias column goes to an fp16 output tile
    (Scalar/Vector alternating), stored to DRAM in per-4-quad groups on
    alternating DMA rings (GpSimd SWDGE / Scalar HWDGE).  fp16 I/O halves
    DMA traffic vs f32; per-core HBM bytes ~4.6 MB -> ~13 us at 360 B/ns.
  * Host scatters the sorted outputs back to original token order.

Layouts (core k, quad q, band r, expert e = chunks[q][4k + r]):
  xt [128, TOTW]  xt[32r+i, X[q] + t] = x[token t of e, feat i]   (fp16)
  wp [128, 1024]  wp[32r+i, 32q + o]  = w[e, i, o]                (fp16)
  bp [128, 32]    bp[32r+o, q]        = b[e, 0, o]                (f32)
  idm [128, 32]   idm[32r+i, j]       = (i == j)                  (fp16)
  ot [128, TOTW]  ot[32r+o, X[q] + t] = out[token t of e, feat o] (fp16)
"""

import numpy as np

import concourse.bass as bass
import concourse.mybir as mybir
import concourse.tile as tile
from concourse import bacc
from concourse.bass_utils import run_bass_kernel_spmd

N_TOK = 262144
E = 1024
F = 32
O = 32
NCORES = 8
NQUAD = 32  # quads per core; 4 experts each = 128 experts/core
GQ = 4  # quads per load/store group
NG = NQUAD // GQ
F32 = mybir.dt.float32
MM_DT = mybir.dt.float16
OT_DT = mybir.dt.float16

N_WARM = 10  # PE ramp warm-up matmuls
WARM_N = 256  # free-dim length of each warm-up matmul

_programs: dict[tuple, "bacc.Bacc"] = {}


class _CapacityOverflow(Exception):
    """A single expert got >512 tokens (~16 sigma out for uniform routing at
    256 tokens/expert).  Handled by a host fallback so kernel() still
    returns a correct result."""


def _plan(counts):
    """Chunk experts into count-matched quads; per-quad widths and offsets."""
    order_e = np.argsort(counts, kind="stable")  # ascending counts
    chunks = order_e.reshape(NQUAD, 32)  # chunk q: ranks [32q, 32q+32)
    Q = np.maximum(16, ((counts[chunks[:, -1]] + 7) // 8) * 8)  # [NQUAD]
    if Q.max() > 512:
        raise _CapacityOverflow(int(counts.max()))
    X = np.zeros(NQUAD + 1, dtype=np.int64)
    np.cumsum(Q, out=X[1:])
    TOTW = int(X[-1])
    j = np.arange(32)
    e_quad = np.empty(E, dtype=np.int64)
    e_core = np.empty(E, dtype=np.int64)
    e_band = np.empty(E, dtype=np.int64)
    e_quad[chunks] = np.arange(NQUAD)[:, None]
    e_core[chunks] = (j // 4)[None, :]
    e_band[chunks] = (j % 4)[None, :]
    return Q.astype(np.int64), X, TOTW, e_quad, e_core, e_band


def _build(Q, X, TOTW) -> "bacc.Bacc":
    nc = bacc.Bacc("TRN2", target_bir_lowering=False, debug=False, num_devices=NCORES)
    xt = nc.declare_dram_parameter("xt", [128, TOTW], MM_DT, isOutput=False)
    wp = nc.declare_dram_parameter("wp", [128, NQUAD * O], MM_DT, isOutput=False)
    bp = nc.declare_dram_parameter("bp", [128, NQUAD], F32, isOutput=False)
    idm = nc.declare_dram_parameter("idm", [128, 32], MM_DT, isOutput=False)
    ot = nc.declare_dram_parameter("ot", [128, TOTW], OT_DT, isOutput=True)

    with tile.TileContext(nc) as tc:
        with (
            tc.tile_pool(name="const", bufs=1) as c_pool,
            tc.tile_pool(name="xt", bufs=3) as xt_pool,
            tc.tile_pool(name="out", bufs=3) as out_pool,
            tc.tile_pool(name="psb", bufs=1, space="PSUM") as psb_pool,
            tc.tile_pool(name="psm", bufs=4, space="PSUM") as psm_pool,
            tc.tile_pool(name="warm", bufs=1, space="PSUM") as warm_pool,
        ):
            wp_t = c_pool.tile([128, NQUAD * O], MM_DT)
            bp_t = c_pool.tile([128, NQUAD], F32)
            id_t = c_pool.tile([128, 32], MM_DT)
            wq_t = c_pool.tile([128, NQUAD * 128], MM_DT)  # 32 block-diag lhsT
            warm_t = c_pool.tile([128, WARM_N], MM_DT)

            # loads: sync (SP HWDGE) carries id/wp/xt; gpsimd (SWDGE) bias
            nc.sync.dma_start(out=id_t[:], in_=idm[:])
            wpc = NQUAD * O // 4  # wp column chunk (8 quads' weights)
            nc.sync.dma_start(out=wp_t[:, :wpc], in_=wp[:, :wpc])
            nc.gpsimd.dma_start(out=bp_t[:], in_=bp[:])

            # PE ramp warm-up on a memset scratch tile (PSUM never read)
            nc.vector.memset(warm_t[:], 0.0)
            warm_ps = warm_pool.tile([128, WARM_N], F32, space="PSUM")
            for _ in range(N_WARM):
                nc.tensor.matmul(
                    out=warm_ps[0:32, :],
                    lhsT=warm_t[0:32, 0:32],
                    rhs=warm_t[0:32, :],
                    start=True,
                    stop=True,
                    tile_position=(0, 0),
                )

            # two fixed build regions; off-diagonal stays zero forever
            psb = [
                psb_pool.tile(
                    [128, 128], F32, space="PSUM", name=f"psb{i}", tag=f"psb{i}"
                )
                for i in range(2)
            ]
            nc.vector.memset(psb[0][:], 0.0)
            nc.vector.memset(psb[1][:], 0.0)

            xt_tiles = {}
            o_tiles = {}

            def load_group(g):
                a, bnd = int(X[GQ * g]), int(X[GQ * (g + 1)])
                t = xt_pool.tile([128, bnd - a], MM_DT, name="xt_t", tag="xt_t")
                nc.sync.dma_start(out=t[:], in_=xt[:, a:bnd])
                xt_tiles[g] = t

            def emit_diag(q):
                # block-diagonal build: psb diag <- wp quad blocks
                if q % 8 == 0 and q // 8 + 1 < 4:
                    c = (q // 8 + 1) * wpc
                    nc.sync.dma_start(out=wp_t[:, c : c + wpc], in_=wp[:, c : c + wpc])
                if q % GQ == 0 and q // GQ + 2 < NG:
                    load_group(q // GQ + 2)
                p = psb[q % 2]
                for r in range(4):
                    nc.tensor.matmul(
                        out=p[32 * r : 32 * r + 32, 32 * r : 32 * r + 32],
                        lhsT=id_t[32 * r : 32 * r + 32, :],
                        rhs=wp_t[32 * r : 32 * r + 32, 32 * q : 32 * q + 32],
                        start=True,
                        stop=True,
                        tile_position=(32 * r, 32 * r),
                    )
                if q % 2 == 0:
                    nc.scalar.activation(
                        wq_t[:, 128 * q : 128 * q + 128],
                        p[:],
                        mybir.ActivationFunctionType.Copy,
                    )
                else:
                    nc.vector.tensor_scalar_add(
                        wq_t[:, 128 * q : 128 * q + 128], p[:], 0.0
                    )

            def emit_main(q):
                g, qi = q // GQ, q % GQ
                if qi == 0:
                    a, bnd = int(X[GQ * g]), int(X[GQ * (g + 1)])
                    o_tiles[g] = out_pool.tile(
                        [128, bnd - a], OT_DT, name="o_t", tag="o_t"
                    )
                Qq = int(Q[q])
                off = int(X[q] - X[GQ * g])
                psm = psm_pool.tile([128, Qq], F32, space="PSUM", name="psm", tag="psm")
                nc.tensor.matmul(
                    out=psm[:],
                    lhsT=wq_t[:, 128 * q : 128 * q + 128],
                    rhs=xt_tiles[g][:, off : off + Qq],
                    start=True,
                    stop=True,
                )
                bias_ap = bp_t[:, q : q + 1]
                o_t = o_tiles[g]
                if q % 2 == 0:
                    nc.vector.tensor_scalar_add(o_t[:, off : off + Qq], psm[:], bias_ap)
                else:
                    nc.scalar.activation(
                        o_t[:, off : off + Qq],
                        psm[:],
                        mybir.ActivationFunctionType.Identity,
                        bias=bias_ap,
                        scale=1.0,
                    )
                if qi == GQ - 1:
                    a, bnd = int(X[GQ * g]), int(X[GQ * (g + 1)])
                    ring = nc.gpsimd if g % 2 == 0 else nc.scalar
                    ring.dma_start(out=ot[:, a:bnd], in_=o_t[:])

            load_group(0)
            load_group(1)
            # software pipeline: build quad q+1 while quad q's lhsT copy runs
            emit_diag(0)
            for q in range(NQUAD):
                if q + 1 < NQUAD:
                    emit_diag(q + 1)
                emit_main(q)

    nc.compile()
    return nc


def _pack(x, inds, w, b):
    """Host-side routing: sort tokens by expert, build per-core device arrays."""
    counts = np.bincount(inds, minlength=E)
    Q, X, TOTW, e_quad, e_core, e_band = _plan(counts)

    order = np.argsort(inds, kind="stable")
    sorted_inds = inds[order]
    starts = np.zeros(E, dtype=np.int64)
    np.cumsum(counts[:-1], out=starts[1:])
    slot = np.arange(N_TOK, dtype=np.int64) - starts[sorted_inds]

    k_tok = e_core[sorted_inds]
    r_tok = e_band[sorted_inds]
    col_tok = X[e_quad[sorted_inds]] + slot

    mdt = mybir.dt.np(MM_DT)
    xt_all = np.zeros((NCORES, 4, F, TOTW), dtype=mdt)
    xt_all[k_tok, r_tok, :, col_tok] = x[order].astype(mdt)
    xt = xt_all.reshape(NCORES, 128, TOTW)

    wpn = np.zeros((NCORES, 4, F, NQUAD, O), dtype=mdt)
    wpn[e_core, e_band, :, e_quad, :] = w.astype(mdt)
    wpk = wpn.reshape(NCORES, 128, NQUAD * O)

    bpn = np.zeros((NCORES, 4, O, NQUAD), dtype=np.float32)
    bpn[e_core, e_band, :, e_quad] = b[:, 0, :]
    bpk = bpn.reshape(NCORES, 128, NQUAD)

    idk = np.tile(np.eye(32, dtype=mdt), (4, 1))

    plan = (Q, X, TOTW)
    return plan, order, (k_tok, r_tok, col_tok), xt, wpk, bpk, idk


def _unpack(results, tok_addr, order):
    k_tok, r_tok, col_tok = tok_addr
    ot = np.stack([results[k]["ot"] for k in range(NCORES)])  # [k, 128, TOTW]
    ot4 = ot.reshape(NCORES, 4, O, -1)  # [k, r, o, col]
    out = np.empty((N_TOK, O), dtype=np.float32)
    out[order] = ot4[k_tok, r_tok, :, col_tok]
    return out


def _prepare(x, inds, w, b):
    """Pack inputs and return (nc, in_maps, tok_addr, order)."""
    plan, order, tok_addr, xt, wpk, bpk, idk = _pack(x, inds, w, b)
    Q, X, TOTW = plan
    key = (MM_DT, OT_DT, Q.tobytes())
    nc = _programs.get(key)
    if nc is None:
        nc = _build(Q, X, TOTW)
        _programs[key] = nc
    in_maps = [
        {"xt": xt[k], "wp": wpk[k], "bp": bpk[k], "idm": idk} for k in range(NCORES)
    ]
    return nc, in_maps, tok_addr, order


def kernel(input, inds, w, b):
    x = np.ascontiguousarray(np.asarray(input, dtype=np.float32))
    inds = np.asarray(inds, dtype=np.int32)
    w = np.ascontiguousarray(np.asarray(w, dtype=np.float32))
    b = np.ascontiguousarray(np.asarray(b, dtype=np.float32))
    assert x.shape == (N_TOK, F) and inds.shape == (N_TOK,)
    assert w.shape == (E, F, O) and b.shape == (E, 1, O)

    try:
        nc, in_maps, tok_addr, order = _prepare(x, inds, w, b)
    except _CapacityOverflow:
        return (np.einsum("ni,nio->no", x, w[inds]) + b[inds, 0]).astype(np.float32)

    res = run_bass_kernel_spmd(nc, in_maps, list(range(NCORES)))
    return _unpack(res.results, tok_addr, order)


def last_program():
    """The most recently compiled Bass program (for profiling in test.py)."""
    return next(iter(_programs.values())) if _programs else None


# revision 5
# speedup vs baseline: 1.2730x; 1.2730x over previous
"""MoE routed expert matmul on 8 Trainium2 NeuronCores.

Problem: out[n] = input[n] @ w[inds[n]] + b[inds[n]]
  input [262144, 32] f32, inds [262144] i32 (1024 experts), w [1024, 32, 32], b [1024, 1, 32]

Strategy (K-stacked expert quads; host does routing/layout only — all FLOPs
on device):
  * Host sorts the 1024 experts by global token count (ascending) and chunks
    them into 32 quad-groups of 32 experts with near-equal counts.  Chunk q
    supplies one expert to each (core, band) pair: expert chunks[q][4k + r]
    goes to core k, quad q, band r (r in 0..3).  Every core runs the same
    program over its own 32 quads; quad q's column width Q[q] = max token
    count in the chunk (global max, so the SPMD shapes match), rounded up to
    8.  Count-matched chunks keep padding to a few percent.
  * Activation layout xt [128, TOTW] fp16: token t of (quad q, band r) sits
    at column X[q] + t, rows 32r..32r+32 (its 32 features).  Each column
    carries up to 4 tokens (one per band) — full 128-row density.
  * Weights upload as block-diagonal K=64 stacks (wq, 0.5 MB): for each quad
    and half h, a [64, 64] tile holds experts (q, 2h) and (q, 2h+1) on the
    diagonal.  Two [K=64, M=64, N=Q] matmuls per quad (tile_position (0,0) /
    (64,64)) then compute all 4 bands' tokens — each activation column
    streams through the PE twice instead of 4x (vs per-expert 32x32 tiles),
    and the off-diagonal zeros kill the cross-expert terms.
  * The PSUM result + per-quad bias column goes to an fp16 output tile
    (Scalar/Vector alternating), stored to DRAM in per-4-quad groups on
    alternating DMA rings (GpSimd SWDGE / Scalar HWDGE).  fp16 I/O halves
    DMA traffic vs f32; per-core HBM bytes ~4.8 MB -> ~13.3 us at 360 B/ns.
  * Host scatters the sorted outputs back to original token order.

Layouts (core k, quad q, band r = 2h + s, expert e = chunks[q][4k + r]):
  xt [128, TOTW]  xt[32r+i, X[q] + t]        = x[token t of e, feat i]  (fp16)
  wq [128, 2048]  wq[64h+32s+i, 64q+32s+o]   = w[e, i, o], 0 off-diag   (fp16)
  bp [128, 32]    bp[32r+o, q]               = b[e, 0, o]               (f32)
  ot [128, TOTW]  ot[32r+o, X[q] + t]        = out[token t of e, feat o](fp16)
"""

import numpy as np

import concourse.bass as bass
import concourse.mybir as mybir
import concourse.tile as tile
from concourse import bacc
from concourse.bass_utils import run_bass_kernel_spmd

N_TOK = 262144
E = 1024
F = 32
O = 32
NCORES = 8
NQUAD = 32  # quads per core; 4 experts each = 128 experts/core
GQ = 4  # quads per load/store group
NG = NQUAD // GQ
F32 = mybir.dt.float32
MM_DT = mybir.dt.float16
OT_DT = mybir.dt.float16

N_WARM = 10  # PE ramp warm-up matmuls
WARM_N = 256  # free-dim length of each warm-up matmul

_programs: dict[tuple, "bacc.Bacc"] = {}


class _CapacityOverflow(Exception):
    """A single expert got >512 tokens (~16 sigma out for uniform routing at
    256 tokens/expert).  Handled by a host fallback so kernel() still
    returns a correct result."""


def _plan(counts):
    """Chunk experts into count-matched quads; per-quad widths and offsets."""
    order_e = np.argsort(counts, kind="stable")  # ascending counts
    chunks = order_e.reshape(NQUAD, 32)  # chunk q: ranks [32q, 32q+32)
    Q = np.maximum(16, ((counts[chunks[:, -1]] + 7) // 8) * 8)  # [NQUAD]
    if Q.max() > 512:
        raise _CapacityOverflow(int(counts.max()))
    X = np.zeros(NQUAD + 1, dtype=np.int64)
    np.cumsum(Q, out=X[1:])
    TOTW = int(X[-1])
    j = np.arange(32)
    e_quad = np.empty(E, dtype=np.int64)
    e_core = np.empty(E, dtype=np.int64)
    e_band = np.empty(E, dtype=np.int64)
    e_quad[chunks] = np.arange(NQUAD)[:, None]
    e_core[chunks] = (j // 4)[None, :]
    e_band[chunks] = (j % 4)[None, :]
    return Q.astype(np.int64), X, TOTW, e_quad, e_core, e_band


def _build(Q, X, TOTW) -> "bacc.Bacc":
    nc = bacc.Bacc("TRN2", target_bir_lowering=False, debug=False, num_devices=NCORES)
    xt = nc.declare_dram_parameter("xt", [128, TOTW], MM_DT, isOutput=False)
    wq = nc.declare_dram_parameter("wq", [128, NQUAD * 64], MM_DT, isOutput=False)
    bp = nc.declare_dram_parameter("bp", [128, NQUAD], F32, isOutput=False)
    ot = nc.declare_dram_parameter("ot", [128, TOTW], OT_DT, isOutput=True)

    with tile.TileContext(nc) as tc:
        with (
            tc.tile_pool(name="const", bufs=1) as c_pool,
            tc.tile_pool(name="xt", bufs=3) as xt_pool,
            tc.tile_pool(name="out", bufs=3) as out_pool,
            tc.tile_pool(name="psm", bufs=4, space="PSUM") as psm_pool,
            tc.tile_pool(name="warm", bufs=1, space="PSUM") as warm_pool,
        ):
            wq_t = c_pool.tile([128, NQUAD * 64], MM_DT)
            bp_t = c_pool.tile([128, NQUAD], F32)
            warm_t = c_pool.tile([128, WARM_N], MM_DT)

            # loads: sync (SP HWDGE) carries wq/xt; gpsimd (SWDGE) the bias
            wqc = NQUAD * 64 // 4  # wq column chunk (8 quads' weights)
            nc.sync.dma_start(out=wq_t[:, :wqc], in_=wq[:, :wqc])
            nc.gpsimd.dma_start(out=bp_t[:], in_=bp[:])

            # PE ramp warm-up on a memset scratch tile (PSUM never read)
            nc.vector.memset(warm_t[:], 0.0)
            warm_ps = warm_pool.tile([128, WARM_N], F32, space="PSUM")
            for _ in range(N_WARM):
                nc.tensor.matmul(
                    out=warm_ps[0:32, :],
                    lhsT=warm_t[0:32, 0:32],
                    rhs=warm_t[0:32, :],
                    start=True,
                    stop=True,
                    tile_position=(0, 0),
                )

            xt_tiles = {}
            o_tiles = {}

            def load_group(g):
                a, bnd = int(X[GQ * g]), int(X[GQ * (g + 1)])
                t = xt_pool.tile([128, bnd - a], MM_DT, name="xt_t", tag="xt_t")
                nc.sync.dma_start(out=t[:], in_=xt[:, a:bnd])
                xt_tiles[g] = t

            load_group(0)
            load_group(1)

            for q in range(NQUAD):
                if q % 8 == 0 and q // 8 + 1 < 4:
                    c = (q // 8 + 1) * wqc
                    nc.sync.dma_start(out=wq_t[:, c : c + wqc], in_=wq[:, c : c + wqc])
                if q % GQ == 0 and q // GQ + 2 < NG:
                    load_group(q // GQ + 2)
                g, qi = q // GQ, q % GQ
                if qi == 0:
                    a, bnd = int(X[GQ * g]), int(X[GQ * (g + 1)])
                    o_tiles[g] = out_pool.tile(
                        [128, bnd - a], OT_DT, name="o_t", tag="o_t"
                    )
                Qq = int(Q[q])
                off = int(X[q] - X[GQ * g])
                psm = psm_pool.tile([128, Qq], F32, space="PSUM", name="psm", tag="psm")
                for h in range(2):
                    nc.tensor.matmul(
                        out=psm[64 * h : 64 * h + 64, :],
                        lhsT=wq_t[64 * h : 64 * h + 64, 64 * q : 64 * q + 64],
                        rhs=xt_tiles[g][64 * h : 64 * h + 64, off : off + Qq],
                        start=True,
                        stop=True,
                        tile_position=(64 * h, 64 * h),
                    )
                bias_ap = bp_t[:, q : q + 1]
                o_t = o_tiles[g]
                if q % 2 == 0:
                    nc.vector.tensor_scalar_add(o_t[:, off : off + Qq], psm[:], bias_ap)
                else:
                    nc.scalar.activation(
                        o_t[:, off : off + Qq],
                        psm[:],
                        mybir.ActivationFunctionType.Identity,
                        bias=bias_ap,
                        scale=1.0,
                    )
                if qi == GQ - 1:
                    a, bnd = int(X[GQ * g]), int(X[GQ * (g + 1)])
                    ring = nc.gpsimd if g % 2 == 0 else nc.scalar
                    ring.dma_start(out=ot[:, a:bnd], in_=o_t[:])

    nc.compile()
    return nc


def _pack(x, inds, w, b):
    """Host-side routing: sort tokens by expert, build per-core device arrays."""
    counts = np.bincount(inds, minlength=E)
    Q, X, TOTW, e_quad, e_core, e_band = _plan(counts)

    order = np.argsort(inds, kind="stable")
    sorted_inds = inds[order]
    starts = np.zeros(E, dtype=np.int64)
    np.cumsum(counts[:-1], out=starts[1:])
    slot = np.arange(N_TOK, dtype=np.int64) - starts[sorted_inds]

    k_tok = e_core[sorted_inds]
    r_tok = e_band[sorted_inds]
    col_tok = X[e_quad[sorted_inds]] + slot

    mdt = mybir.dt.np(MM_DT)
    xt_all = np.zeros((NCORES, 4, F, TOTW), dtype=mdt)
    xt_all[k_tok, r_tok, :, col_tok] = x[order].astype(mdt)
    xt = xt_all.reshape(NCORES, 128, TOTW)

    # wq[k, h, s, i, q, s', o] = w[e, i, o] on the s == s' diagonal
    e_half = e_band // 2
    e_sub = e_band % 2
    wqn = np.zeros((NCORES, 2, 2, F, NQUAD, 2, O), dtype=mdt)
    wqn[e_core, e_half, e_sub, :, e_quad, e_sub, :] = w.astype(mdt)
    wqk = wqn.reshape(NCORES, 128, NQUAD * 64)

    bpn = np.zeros((NCORES, 4, O, NQUAD), dtype=np.float32)
    bpn[e_core, e_band, :, e_quad] = b[:, 0, :]
    bpk = bpn.reshape(NCORES, 128, NQUAD)

    plan = (Q, X, TOTW)
    return plan, order, (k_tok, r_tok, col_tok), xt, wqk, bpk


def _unpack(results, tok_addr, order):
    k_tok, r_tok, col_tok = tok_addr
    ot = np.stack([results[k]["ot"] for k in range(NCORES)])  # [k, 128, TOTW]
    ot4 = ot.reshape(NCORES, 4, O, -1)  # [k, r, o, col]
    out = np.empty((N_TOK, O), dtype=np.float32)
    out[order] = ot4[k_tok, r_tok, :, col_tok]
    return out


def _prepare(x, inds, w, b):
    """Pack inputs and return (nc, in_maps, tok_addr, order)."""
    plan, order, tok_addr, xt, wqk, bpk = _pack(x, inds, w, b)
    Q, X, TOTW = plan
    key = (MM_DT, OT_DT, Q.tobytes())
    nc = _programs.get(key)
    if nc is None:
        nc = _build(Q, X, TOTW)
        _programs[key] = nc
    in_maps = [{"xt": xt[k], "wq": wqk[k], "bp": bpk[k]} for k in range(NCORES)]
    return nc, in_maps, tok_addr, order


def kernel(input, inds, w, b):
    x = np.ascontiguousarray(np.asarray(input, dtype=np.float32))
    inds = np.asarray(inds, dtype=np.int32)
    w = np.ascontiguousarray(np.asarray(w, dtype=np.float32))
    b = np.ascontiguousarray(np.asarray(b, dtype=np.float32))
    assert x.shape == (N_TOK, F) and inds.shape == (N_TOK,)
    assert w.shape == (E, F, O) and b.shape == (E, 1, O)

    try:
        nc, in_maps, tok_addr, order = _prepare(x, inds, w, b)
    except _CapacityOverflow:
        return (np.einsum("ni,nio->no", x, w[inds]) + b[inds, 0]).astype(np.float32)

    res = run_bass_kernel_spmd(nc, in_maps, list(range(NCORES)))
    return _unpack(res.results, tok_addr, order)


def last_program():
    """The most recently compiled Bass program (for profiling in test.py)."""
    return next(iter(_programs.values())) if _programs else None


# revision 10
# speedup vs baseline: 1.3030x; 1.0236x over previous
"""MoE routed expert matmul on 8 Trainium2 NeuronCores.

Problem: out[n] = input[n] @ w[inds[n]] + b[inds[n]]
  input [262144, 32] f32, inds [262144] i32 (1024 experts), w [1024, 32, 32], b [1024, 1, 32]

Strategy (K-stacked expert quads; host does routing/layout only — all FLOPs
on device):
  * Host sorts the 1024 experts by global token count (ascending) and chunks
    them into 32 quad-groups of 32 experts with near-equal counts.  Chunk q
    supplies one expert to each (core, band) pair: expert chunks[q][4k + r]
    goes to core k, quad q, band r (r in 0..3).  Every core runs the same
    program over its own 32 quads; quad q's column width Q[q] = max token
    count in the chunk (global max, so the SPMD shapes match), rounded up to
    8.  Count-matched chunks keep padding to a few percent.
  * Activation layout xt [128, TOTW] fp16: token t of (quad q, band r) sits
    at column X[q] + t, rows 32r..32r+32 (its 32 features).  Each column
    carries up to 4 tokens (one per band) — full 128-row density.
  * Weights upload as block-diagonal K=64 stacks (wq, 0.5 MB): for each quad
    and half h, a [64, 64] tile holds experts (q, 2h) and (q, 2h+1) on the
    diagonal.  Two [K=64, M=64, N=Q] matmuls per quad (tile_position (0,0) /
    (64,64)) then compute all 4 bands' tokens — each activation column
    streams through the PE twice instead of 4x (vs per-expert 32x32 tiles),
    and the off-diagonal zeros kill the cross-expert terms.
  * The PSUM result + per-quad bias column goes to an fp16 output tile
    (Scalar/Vector alternating), stored to DRAM in per-4-quad groups on
    alternating DMA rings (GpSimd SWDGE / Scalar HWDGE).  fp16 I/O halves
    DMA traffic vs f32; per-core HBM bytes ~4.8 MB -> ~13.3 us at 360 B/ns.
  * Host scatters the sorted outputs back to original token order.

Layouts (core k, quad q, band r = 2h + s, expert e = chunks[q][4k + r]):
  xt [128, TOTW]  xt[32r+i, X[q] + t]        = x[token t of e, feat i]  (fp16)
  wq [128, 2048]  wq[64h+32s+i, 64q+32s+o]   = w[e, i, o], 0 off-diag   (fp16)
  bp [128, 32]    bp[32r+o, q]               = b[e, 0, o]               (f32)
  ot [128, TOTW]  ot[32r+o, X[q] + t]        = out[token t of e, feat o](fp16)
"""

import numpy as np

import concourse.bass as bass
import concourse.mybir as mybir
import concourse.tile as tile
from concourse import bacc
from concourse.bass_utils import run_bass_kernel_spmd

N_TOK = 262144
E = 1024
F = 32
O = 32
NCORES = 8
NQUAD = 32  # quads per core; 4 experts each = 128 experts/core
GQ = 4  # quads per load/store group
NG = NQUAD // GQ
F32 = mybir.dt.float32
MM_DT = mybir.dt.float16
OT_DT = mybir.dt.float16

N_WARM = 8  # PE ramp warm-up matmuls
WARM_N = 160  # free-dim length of each warm-up matmul

_programs: dict[tuple, "bacc.Bacc"] = {}


class _CapacityOverflow(Exception):
    """A single expert got >512 tokens (~16 sigma out for uniform routing at
    256 tokens/expert).  Handled by a host fallback so kernel() still
    returns a correct result."""


def _plan(counts):
    """Chunk experts into count-matched quads; per-quad widths and offsets."""
    order_e = np.argsort(counts, kind="stable")  # ascending counts
    # chunk q holds 32 count-matched experts; descending so the pipeline
    # tail (last-stored groups) drains on the smallest transfers
    chunks = order_e.reshape(NQUAD, 32)[::-1]
    Q = np.maximum(16, ((counts[chunks[:, -1]] + 7) // 8) * 8)  # [NQUAD]
    if Q.max() > 512:
        raise _CapacityOverflow(int(counts.max()))
    X = np.zeros(NQUAD + 1, dtype=np.int64)
    np.cumsum(Q, out=X[1:])
    TOTW = int(X[-1])
    j = np.arange(32)
    e_quad = np.empty(E, dtype=np.int64)
    e_core = np.empty(E, dtype=np.int64)
    e_band = np.empty(E, dtype=np.int64)
    e_quad[chunks] = np.arange(NQUAD)[:, None]
    e_core[chunks] = (j // 4)[None, :]
    e_band[chunks] = (j % 4)[None, :]
    return Q.astype(np.int64), X, TOTW, e_quad, e_core, e_band


def _build(Q, X, TOTW) -> "bacc.Bacc":
    nc = bacc.Bacc("TRN2", target_bir_lowering=False, debug=False, num_devices=NCORES)
    xt = nc.declare_dram_parameter("xt", [128, TOTW], MM_DT, isOutput=False)
    wq = nc.declare_dram_parameter("wq", [128, NQUAD * 64], MM_DT, isOutput=False)
    bp = nc.declare_dram_parameter("bp", [128, NQUAD], F32, isOutput=False)
    ot = nc.declare_dram_parameter("ot", [128, TOTW], OT_DT, isOutput=True)

    with tile.TileContext(nc) as tc:
        with (
            tc.tile_pool(name="const", bufs=1) as c_pool,
            tc.tile_pool(name="xt", bufs=4) as xt_pool,
            tc.tile_pool(name="out", bufs=4) as out_pool,
            tc.tile_pool(name="psm", bufs=6, space="PSUM") as psm_pool,
            tc.tile_pool(name="warm", bufs=1, space="PSUM") as warm_pool,
        ):
            wq_t = c_pool.tile([128, NQUAD * 64], MM_DT)
            bp_t = c_pool.tile([128, NQUAD], F32)
            warm_t = c_pool.tile([128, WARM_N], MM_DT)

            # loads: sync (SP HWDGE) carries the first wq half + all xt;
            # gpsimd (SWDGE) the second wq half and the bias
            wqh = NQUAD * 64 // 2
            nc.sync.dma_start(out=wq_t[:, :wqh], in_=wq[:, :wqh])
            nc.gpsimd.dma_start(out=wq_t[:, wqh:], in_=wq[:, wqh:])
            nc.gpsimd.dma_start(out=bp_t[:], in_=bp[:])

            # PE ramp warm-up on a memset scratch tile (PSUM never read)
            nc.vector.memset(warm_t[:], 0.0)
            warm_ps = warm_pool.tile([128, WARM_N], F32, space="PSUM")
            for _ in range(N_WARM):
                nc.tensor.matmul(
                    out=warm_ps[0:32, :],
                    lhsT=warm_t[0:32, 0:32],
                    rhs=warm_t[0:32, :],
                    start=True,
                    stop=True,
                    tile_position=(0, 0),
                )

            xt_tiles = {}
            o_tiles = {}

            def load_group(g, split=1):
                a, bnd = int(X[GQ * g]), int(X[GQ * (g + 1)])
                t = xt_pool.tile([128, bnd - a], MM_DT, name="xt_t", tag="xt_t")
                w_ = bnd - a
                for s in range(split):
                    c0, c1 = s * w_ // split, (s + 1) * w_ // split
                    nc.sync.dma_start(out=t[:, c0:c1], in_=xt[:, a + c0 : a + c1])
                xt_tiles[g] = t

            load_group(0, split=2)
            load_group(1)

            for q in range(NQUAD):
                if q % GQ == 0 and q // GQ + 2 < NG:
                    load_group(q // GQ + 2)
                g, qi = q // GQ, q % GQ
                if qi == 0:
                    a, bnd = int(X[GQ * g]), int(X[GQ * (g + 1)])
                    o_tiles[g] = out_pool.tile(
                        [128, bnd - a], OT_DT, name="o_t", tag="o_t"
                    )
                Qq = int(Q[q])
                off = int(X[q] - X[GQ * g])
                psm = psm_pool.tile([128, Qq], F32, space="PSUM", name="psm", tag="psm")
                for h in range(2):
                    nc.tensor.matmul(
                        out=psm[64 * h : 64 * h + 64, :],
                        lhsT=wq_t[64 * h : 64 * h + 64, 64 * q : 64 * q + 64],
                        rhs=xt_tiles[g][64 * h : 64 * h + 64, off : off + Qq],
                        start=True,
                        stop=True,
                        tile_position=(64 * h, 64 * h),
                    )
                bias_ap = bp_t[:, q : q + 1]
                o_t = o_tiles[g]
                if q % 2 == 0:
                    nc.vector.tensor_scalar_add(o_t[:, off : off + Qq], psm[:], bias_ap)
                else:
                    nc.scalar.activation(
                        o_t[:, off : off + Qq],
                        psm[:],
                        mybir.ActivationFunctionType.Identity,
                        bias=bias_ap,
                        scale=1.0,
                    )
                if qi == GQ - 1:
                    a, bnd = int(X[GQ * g]), int(X[GQ * (g + 1)])
                    ring = nc.gpsimd if g % 2 == 0 else nc.scalar
                    ring.dma_start(out=ot[:, a:bnd], in_=o_t[:])

    nc.compile()
    return nc


def _pack(x, inds, w, b):
    """Host-side routing: sort tokens by expert, build per-core device arrays."""
    counts = np.bincount(inds, minlength=E)
    Q, X, TOTW, e_quad, e_core, e_band = _plan(counts)

    order = np.argsort(inds, kind="stable")
    sorted_inds = inds[order]
    starts = np.zeros(E, dtype=np.int64)
    np.cumsum(counts[:-1], out=starts[1:])
    slot = np.arange(N_TOK, dtype=np.int64) - starts[sorted_inds]

    k_tok = e_core[sorted_inds]
    r_tok = e_band[sorted_inds]
    col_tok = X[e_quad[sorted_inds]] + slot

    mdt = mybir.dt.np(MM_DT)
    xt_all = np.zeros((NCORES, 4, F, TOTW), dtype=mdt)
    xt_all[k_tok, r_tok, :, col_tok] = x[order].astype(mdt)
    xt = xt_all.reshape(NCORES, 128, TOTW)

    # wq[k, h, s, i, q, s', o] = w[e, i, o] on the s == s' diagonal
    e_half = e_band // 2
    e_sub = e_band % 2
    wqn = np.zeros((NCORES, 2, 2, F, NQUAD, 2, O), dtype=mdt)
    wqn[e_core, e_half, e_sub, :, e_quad, e_sub, :] = w.astype(mdt)
    wqk = wqn.reshape(NCORES, 128, NQUAD * 64)

    bpn = np.zeros((NCORES, 4, O, NQUAD), dtype=np.float32)
    bpn[e_core, e_band, :, e_quad] = b[:, 0, :]
    bpk = bpn.reshape(NCORES, 128, NQUAD)

    plan = (Q, X, TOTW)
    return plan, order, (k_tok, r_tok, col_tok), xt, wqk, bpk


def _unpack(results, tok_addr, order):
    k_tok, r_tok, col_tok = tok_addr
    ot = np.stack([results[k]["ot"] for k in range(NCORES)])  # [k, 128, TOTW]
    ot4 = ot.reshape(NCORES, 4, O, -1)  # [k, r, o, col]
    out = np.empty((N_TOK, O), dtype=np.float32)
    out[order] = ot4[k_tok, r_tok, :, col_tok]
    return out


def _prepare(x, inds, w, b):
    """Pack inputs and return (nc, in_maps, tok_addr, order)."""
    plan, order, tok_addr, xt, wqk, bpk = _pack(x, inds, w, b)
    Q, X, TOTW = plan
    key = (MM_DT, OT_DT, Q.tobytes())
    nc = _programs.get(key)
    if nc is None:
        nc = _build(Q, X, TOTW)
        _programs[key] = nc
    in_maps = [{"xt": xt[k], "wq": wqk[k], "bp": bpk[k]} for k in range(NCORES)]
    return nc, in_maps, tok_addr, order


def kernel(input, inds, w, b):
    x = np.ascontiguousarray(np.asarray(input, dtype=np.float32))
    inds = np.asarray(inds, dtype=np.int32)
    w = np.ascontiguousarray(np.asarray(w, dtype=np.float32))
    b = np.ascontiguousarray(np.asarray(b, dtype=np.float32))
    assert x.shape == (N_TOK, F) and inds.shape == (N_TOK,)
    assert w.shape == (E, F, O) and b.shape == (E, 1, O)

    try:
        nc, in_maps, tok_addr, order = _prepare(x, inds, w, b)
    except _CapacityOverflow:
        return (np.einsum("ni,nio->no", x, w[inds]) + b[inds, 0]).astype(np.float32)

    res = run_bass_kernel_spmd(nc, in_maps, list(range(NCORES)))
    return _unpack(res.results, tok_addr, order)


def last_program():
    """The most recently compiled Bass program (for profiling in test.py)."""
    return next(iter(_programs.values())) if _programs else None
